# revision 12
# baseline (speedup 1.0000x reference)
"""Trainium2 Bass kernel for nn_NonSpikingOutput.

Reference semantics (N=4096 neurons, O=3 outputs, T=4096 steps):
    g = k/(e-k); act = clip(u, 0, 1); RK2 with i_syn frozen collapses to
        v_t = a_t * v_{t-1} + b_t
        a_t = 0.625 - 0.075*act*g,  b_t = 0.075*act*g*e = (0.625 - a_t)*e
    out[o, t] = sum_n v[n, o, t]

v6 design (from HW microbenchmarks):
  - Inputs uploaded as bf16 (host truncation): halves HBM traffic and makes
    every DVE tensor_tensor eligible for the 2x perf mode.
  - d = e-k computed on the PE: psum_d = I@e + (-I)@k (identity stationaries
    uploaded as host constants). ACT Ln reads the f32 psum directly.
  - 0.075 folded into the ACT Exp bias: h = exp(-ln(e-k) + ln 0.075)
    = 0.075/(e-k), so c = act*k*h, a = 0.625 - c (ACT Copy), b = c*e (DVE),
    and the scan yields v directly -- no rescale.
  - DVE carries only: clip (TS 4x), t=k*h, c=t*act, b=c*e (TT 2x bf16), scan.
  - Software pipelining: scan/carry/colsum for tile i-1 are emitted during
    tile i, so the c(i)->a(i)->scan(i) ACT round trip never stalls DVE.
  - PSUM: d tile (128,2048)f32 = 4 banks (bufs=1) + colsum row (1,2048)f32 =
    4 banks (bufs=1) -- exactly 8 banks.
  - No GPSIMD (concurrent GPSIMD inflates DVE TT 4.5x via SBUF port sharing).

Sharding: neuron dim N split across 8 cores (512 each); host sums the
per-core (O, T) partials.
"""

import sys
from contextlib import ExitStack

import numpy as np

sys.path.insert(0, "/opt/trn_rl_repo")

import concourse.bass as bass
import concourse.tile as tile
from concourse import bacc, mybir
from concourse.bass_utils import run_bass_kernel_spmd

N_CORES = 8
N, O, T = 4096, 3, 4096
NL = N // N_CORES  # neurons per core
NG = NL // 128     # 128-partition neuron groups per core
F = 2048           # time-chunk (free dim) per tile
TC = T // F
FP32 = mybir.dt.float32
BF16 = mybir.dt.bfloat16
OP = mybir.AluOpType
AF = mybir.ActivationFunctionType

LN_0075 = float(np.log(0.075))  # Exp bias: exp(-lnd + ln 0.075) = 0.075/d


def _build_nc() -> bass.Bass:
    nc = bacc.Bacc(
        "TRN2", target_bir_lowering=False, debug=False, num_devices=N_CORES
    )
    u = nc.dram_tensor("u", [NL, T], BF16, kind="ExternalInput")
    k = nc.dram_tensor("k", [NL, O, T], BF16, kind="ExternalInput")
    e = nc.dram_tensor("e", [NL, O, T], BF16, kind="ExternalInput")
    ident_d = nc.dram_tensor("ident", [128, 128], BF16, kind="ExternalInput")
    nident_d = nc.dram_tensor("nident", [128, 128], BF16, kind="ExternalInput")
    out = nc.dram_tensor("out", [O, T], FP32, kind="ExternalOutput")

    with tile.TileContext(nc) as tc, ExitStack() as ctx:
        # Preload the ACT table set holding Ln+Exp+Copy (set 6) once.
        preload = mybir.InstLoadActFuncSet(
            name=nc.get_next_instruction_name(), act_func_set_id=6, ins=[], outs=[]
        )
        nc.scalar.add_instruction(preload)

        const_pool = ctx.enter_context(tc.tile_pool(name="const", bufs=1))
        ones = const_pool.tile([128, 1], BF16)
        nc.vector.memset(ones[:], 1.0)
        exp_bias = const_pool.tile([128, 1], FP32)
        nc.vector.memset(exp_bias[:], LN_0075)
        ident = const_pool.tile([128, 128], BF16)
        nc.sync.dma_start(ident[:], ident_d[:, :])
        nident = const_pool.tile([128, 128], BF16)
        nc.sync.dma_start(nident[:], nident_d[:, :])
        # one carry column per (o, g): column o*NG+g
        carry = const_pool.tile([128, O * NG], FP32)

        u_pool = ctx.enter_context(tc.tile_pool(name="u", bufs=2))
        act_pool = ctx.enter_context(tc.tile_pool(name="act", bufs=TC * NG + 1))
        k_pool = ctx.enter_context(tc.tile_pool(name="k", bufs=3))
        e_pool = ctx.enter_context(tc.tile_pool(name="e", bufs=3))
        l_pool = ctx.enter_context(tc.tile_pool(name="l", bufs=2))
        h_pool = ctx.enter_context(tc.tile_pool(name="h", bufs=2))
        t_pool = ctx.enter_context(tc.tile_pool(name="t", bufs=2))
        c_pool = ctx.enter_context(tc.tile_pool(name="c", bufs=2))
        a_pool = ctx.enter_context(tc.tile_pool(name="a", bufs=3))
        b_pool = ctx.enter_context(tc.tile_pool(name="b", bufs=3))
        w_pool = ctx.enter_context(tc.tile_pool(name="w", bufs=2))
        r_pool = ctx.enter_context(tc.tile_pool(name="r", bufs=2))
        ps_pool = ctx.enter_context(tc.tile_pool(name="ps", bufs=1, space="PSUM"))
        d_pool = ctx.enter_context(tc.tile_pool(name="d", bufs=1, space="PSUM"))

        acts: dict[tuple, object] = {}
        # Prefetch all u tiles and run the clips up front: fills the DVE
        # during the DMA/PE/ACT warmup of the first k/e tiles.
        for tci in range(TC):
            for g in range(NG):
                p0, t0 = g * 128, tci * F
                ut = u_pool.tile([128, F], BF16, tag="u")
                nc.sync.dma_start(ut[:], u[p0 : p0 + 128, t0 : t0 + F])
                av = act_pool.tile([128, F], BF16, tag="act")
                nc.vector.tensor_scalar(av[:], ut[:], 0.0, 1.0, OP.max, OP.min)
                acts[(tci, g)] = av

        ps_by_to: dict[tuple, object] = {}
        pending = None  # (tci, o, g, at, bt)

        def emit_tail(item):
            """scan + carry + colsum for a finished front-end tile.

            The scan runs in two F/2 halves chained via the init AP so it can
            start as soon as the first half of `a` is ready on ACT.
            """
            tci, o, g, at, bt = item
            H = F // 2
            wt = w_pool.tile([128, F], BF16, tag="w")
            ci = o * NG + g
            init = 0.0 if tci == 0 else carry[:, ci : ci + 1]
            nc.vector.tensor_tensor_scan(
                wt[:, 0:H], at[:, 0:H], bt[:, 0:H], init, OP.mult, OP.add
            )
            nc.vector.tensor_tensor_scan(
                wt[:, H:F],
                at[:, H:F],
                bt[:, H:F],
                wt[:, H - 1 : H],
                OP.mult,
                OP.add,
            )
            if tci < TC - 1:
                nc.scalar.copy(carry[:, ci : ci + 1], wt[:, F - 1 : F])
            if g == 0:
                ps_by_to[(tci, o)] = ps_pool.tile(
                    [1, F], FP32, tag="ps", name=f"ps{tci}_{o}"
                )
            ps = ps_by_to[(tci, o)]
            for s in range(F // 512):
                nc.tensor.matmul(
                    ps[0:1, s * 512 : (s + 1) * 512],
                    ones[:],
                    wt[:, s * 512 : (s + 1) * 512],
                    start=(g == 0),
                    stop=(g == NG - 1),
                )
            if g == NG - 1:
                t0 = tci * F
                row = r_pool.tile([1, F], FP32, tag="row")
                nc.scalar.copy(row[:], ps[:])
                nc.sync.dma_start(out[o : o + 1, t0 : t0 + F], row[:, :])

        for tci in range(TC):
            t0 = tci * F
            for o in range(O):
                for g in range(NG):
                    p0 = g * 128
                    act = acts[(tci, g)]

                    kt = k_pool.tile([128, F], BF16, tag="k")
                    nc.sync.dma_start(kt[:], k[p0 : p0 + 128, o, t0 : t0 + F])
                    et = e_pool.tile([128, F], BF16, tag="e")
                    nc.sync.dma_start(et[:], e[p0 : p0 + 128, o, t0 : t0 + F])

                    # d = e - k on the PE: per 512-chunk, I@e then (-I)@k
                    dps = d_pool.tile([128, F], FP32, tag="d", name=f"d{tci}_{o}_{g}")
                    for s in range(F // 512):
                        sl = slice(s * 512, (s + 1) * 512)
                        nc.tensor.matmul(
                            dps[:, sl], ident[:], et[:, sl], start=True, stop=False
                        )
                        nc.tensor.matmul(
                            dps[:, sl], nident[:], kt[:, sl], start=False, stop=True
                        )

                    lnd = l_pool.tile([128, F], FP32, tag="lnd")
                    nc.scalar.activation(lnd[:], dps[:], AF.Ln)
                    ht = h_pool.tile([128, F], BF16, tag="h")
                    nc.scalar.activation(
                        ht[:], lnd[:], AF.Exp, bias=exp_bias[:], scale=-1.0
                    )

                    tt = t_pool.tile([128, F], BF16, tag="t")
                    nc.vector.tensor_tensor(tt[:], kt[:], ht[:], OP.mult)
                    ct = c_pool.tile([128, F], BF16, tag="c")
                    nc.vector.tensor_tensor(ct[:], tt[:], act[:], OP.mult)
                    at = a_pool.tile([128, F], BF16, tag="a")
                    H = F // 2
                    nc.scalar.activation(
                        at[:, 0:H], ct[:, 0:H], AF.Copy, bias=0.625, scale=-1.0
                    )
                    nc.scalar.activation(
                        at[:, H:F], ct[:, H:F], AF.Copy, bias=0.625, scale=-1.0
                    )
                    bt = b_pool.tile([128, F], BF16, tag="b")
                    nc.vector.tensor_tensor(bt[:], ct[:], et[:], OP.mult)

                    if pending is not None:
                        emit_tail(pending)
                    pending = (tci, o, g, at, bt)
        emit_tail(pending)

    nc.compile()
    return nc


_NC_CACHE: list = []


def _to_bf16(a: np.ndarray) -> np.ndarray:
    import ml_dtypes

    return np.ascontiguousarray(a.astype(ml_dtypes.bfloat16))


def build_in_maps(u_pre: np.ndarray, k_syn: np.ndarray, e_syn: np.ndarray) -> list:
    import ml_dtypes

    eye = np.eye(128, dtype=ml_dtypes.bfloat16)
    neye = (-np.eye(128)).astype(ml_dtypes.bfloat16)
    in_maps = []
    for i in range(N_CORES):
        lo, hi = i * NL, (i + 1) * NL
        in_maps.append(
            {
                "u": _to_bf16(u_pre[lo:hi, 0, :]),
                "k": _to_bf16(k_syn[lo:hi]),
                "e": _to_bf16(e_syn[lo:hi]),
                "ident": eye,
                "nident": neye,
            }
        )
    return in_maps


def kernel(u_pre: np.ndarray, k_syn: np.ndarray, e_syn: np.ndarray) -> np.ndarray:
    if not _NC_CACHE:
        _NC_CACHE.append(_build_nc())
    nc = _NC_CACHE[0]

    in_maps = build_in_maps(u_pre, k_syn, e_syn)
    res = run_bass_kernel_spmd(nc, in_maps, list(range(N_CORES)))
    partials = np.stack([res.results[i]["out"] for i in range(N_CORES)])
    return partials.sum(axis=0, dtype=np.float32)


# revision 14
# speedup vs baseline: 1.0374x; 1.0374x over previous
"""Trainium2 Bass kernel for nn_NonSpikingOutput.

Reference semantics (N=4096 neurons, O=3 outputs, T=4096 steps):
    g = k/(e-k); act = clip(u, 0, 1); RK2 with i_syn frozen collapses to
        v_t = a_t * v_{t-1} + b_t
        a_t = 0.625 - 0.075*act*g,  b_t = 0.075*act*g*e = (0.625 - a_t)*e
    out[o, t] = sum_n v[n, o, t]

v6 design (from HW microbenchmarks):
  - Inputs uploaded as bf16 (host truncation): halves HBM traffic and makes
    every DVE tensor_tensor eligible for the 2x perf mode.
  - d = e-k computed on the PE: psum_d = I@e + (-I)@k (identity stationaries
    uploaded as host constants). ACT Ln reads the f32 psum directly.
  - 0.075 folded into the ACT Exp bias: h = exp(-ln(e-k) + ln 0.075)
    = 0.075/(e-k), so c = act*k*h, a = 0.625 - c (ACT Copy), b = c*e (DVE),
    and the scan yields v directly -- no rescale.
  - DVE carries only: clip (TS 4x), t=k*h, c=t*act, b=c*e (TT 2x bf16), scan.
  - Software pipelining: scan/carry/colsum for tile i-1 are emitted during
    tile i, so the c(i)->a(i)->scan(i) ACT round trip never stalls DVE.
  - PSUM: d tile (128,2048)f32 = 4 banks (bufs=1) + colsum row (1,2048)f32 =
    4 banks (bufs=1) -- exactly 8 banks.
  - No GPSIMD (concurrent GPSIMD inflates DVE TT 4.5x via SBUF port sharing).

Sharding: neuron dim N split across 8 cores (512 each); host sums the
per-core (O, T) partials.
"""

import sys
from contextlib import ExitStack

import numpy as np

sys.path.insert(0, "/opt/trn_rl_repo")

import concourse.bass as bass
import concourse.tile as tile
from concourse import bacc, mybir
from concourse.bass_utils import run_bass_kernel_spmd

N_CORES = 8
N, O, T = 4096, 3, 4096
NL = N // N_CORES  # neurons per core
NG = NL // 128     # 128-partition neuron groups per core
F = 2048           # time-chunk (free dim) per tile
TC = T // F
FP32 = mybir.dt.float32
BF16 = mybir.dt.bfloat16
OP = mybir.AluOpType
AF = mybir.ActivationFunctionType

LN_0075 = float(np.log(0.075))  # Exp bias: exp(-lnd + ln 0.075) = 0.075/d


def _build_nc() -> bass.Bass:
    nc = bacc.Bacc(
        "TRN2", target_bir_lowering=False, debug=False, num_devices=N_CORES
    )
    u = nc.dram_tensor("u", [NL, T], BF16, kind="ExternalInput")
    k = nc.dram_tensor("k", [NL, O, T], BF16, kind="ExternalInput")
    e = nc.dram_tensor("e", [NL, O, T], BF16, kind="ExternalInput")
    ident_d = nc.dram_tensor("ident", [128, 128], BF16, kind="ExternalInput")
    nident_d = nc.dram_tensor("nident", [128, 128], BF16, kind="ExternalInput")
    out = nc.dram_tensor("out", [O, T], FP32, kind="ExternalOutput")

    with tile.TileContext(nc) as tc, ExitStack() as ctx:
        # Preload the ACT table set holding Ln+Exp+Copy (set 6) once.
        preload = mybir.InstLoadActFuncSet(
            name=nc.get_next_instruction_name(), act_func_set_id=6, ins=[], outs=[]
        )
        nc.scalar.add_instruction(preload)

        const_pool = ctx.enter_context(tc.tile_pool(name="const", bufs=1))
        ones = const_pool.tile([128, 1], BF16)
        nc.vector.memset(ones[:], 1.0)
        exp_bias = const_pool.tile([128, 1], FP32)
        nc.vector.memset(exp_bias[:], LN_0075)
        ident = const_pool.tile([128, 128], BF16)
        nc.sync.dma_start(ident[:], ident_d[:, :])
        nident = const_pool.tile([128, 128], BF16)
        nc.sync.dma_start(nident[:], nident_d[:, :])
        # one carry column per (o, g): column o*NG+g
        carry = const_pool.tile([128, O * NG], FP32)

        u_pool = ctx.enter_context(tc.tile_pool(name="u", bufs=2))
        act_pool = ctx.enter_context(tc.tile_pool(name="act", bufs=NG + 1))
        k_pool = ctx.enter_context(tc.tile_pool(name="k", bufs=3))
        e_pool = ctx.enter_context(tc.tile_pool(name="e", bufs=3))
        l_pool = ctx.enter_context(tc.tile_pool(name="l", bufs=3))
        h_pool = ctx.enter_context(tc.tile_pool(name="h", bufs=3))
        t_pool = ctx.enter_context(tc.tile_pool(name="t", bufs=3))
        c_pool = ctx.enter_context(tc.tile_pool(name="c", bufs=3))
        a_pool = ctx.enter_context(tc.tile_pool(name="a", bufs=3))
        b_pool = ctx.enter_context(tc.tile_pool(name="b", bufs=3))
        w_pool = ctx.enter_context(tc.tile_pool(name="w", bufs=2))
        r_pool = ctx.enter_context(tc.tile_pool(name="r", bufs=2))
        ps_pool = ctx.enter_context(tc.tile_pool(name="ps", bufs=1, space="PSUM"))
        d_pool = ctx.enter_context(tc.tile_pool(name="d", bufs=1, space="PSUM"))

        acts: dict[int, object] = {}
        ps_by_to: dict[tuple, object] = {}
        pending = None  # (tci, o, g, at, bt)

        def emit_tail(item):
            """scan + carry + colsum for a finished front-end tile."""
            tci, o, g, at, bt = item
            wt = w_pool.tile([128, F], BF16, tag="w")
            ci = o * NG + g
            init = 0.0 if tci == 0 else carry[:, ci : ci + 1]
            nc.vector.tensor_tensor_scan(wt[:], at[:], bt[:], init, OP.mult, OP.add)
            if tci < TC - 1:
                nc.scalar.copy(carry[:, ci : ci + 1], wt[:, F - 1 : F])
            if g == 0:
                ps_by_to[(tci, o)] = ps_pool.tile(
                    [1, F], FP32, tag="ps", name=f"ps{tci}_{o}"
                )
            ps = ps_by_to[(tci, o)]
            for s in range(F // 512):
                nc.tensor.matmul(
                    ps[0:1, s * 512 : (s + 1) * 512],
                    ones[:],
                    wt[:, s * 512 : (s + 1) * 512],
                    start=(g == 0),
                    stop=(g == NG - 1),
                )
            if g == NG - 1:
                t0 = tci * F
                row = r_pool.tile([1, F], FP32, tag="row")
                nc.scalar.copy(row[:], ps[:])
                nc.sync.dma_start(out[o : o + 1, t0 : t0 + F], row[:, :])

        for tci in range(TC):
            t0 = tci * F
            for o in range(O):
                for g in range(NG):
                    p0 = g * 128
                    if o == 0:
                        ut = u_pool.tile([128, F], BF16, tag="u")
                        nc.sync.dma_start(ut[:], u[p0 : p0 + 128, t0 : t0 + F])
                        av = act_pool.tile([128, F], BF16, tag="act")
                        nc.vector.tensor_scalar(av[:], ut[:], 0.0, 1.0, OP.max, OP.min)
                        acts[g] = av
                    act = acts[g]

                    kt = k_pool.tile([128, F], BF16, tag="k")
                    nc.sync.dma_start(kt[:], k[p0 : p0 + 128, o, t0 : t0 + F])
                    et = e_pool.tile([128, F], BF16, tag="e")
                    nc.sync.dma_start(et[:], e[p0 : p0 + 128, o, t0 : t0 + F])

                    # d = e - k on the PE: per 512-chunk, I@e then (-I)@k
                    dps = d_pool.tile([128, F], FP32, tag="d", name=f"d{tci}_{o}_{g}")
                    for s in range(F // 512):
                        sl = slice(s * 512, (s + 1) * 512)
                        nc.tensor.matmul(
                            dps[:, sl], ident[:], et[:, sl], start=True, stop=False
                        )
                        nc.tensor.matmul(
                            dps[:, sl], nident[:], kt[:, sl], start=False, stop=True
                        )

                    lnd = l_pool.tile([128, F], FP32, tag="lnd")
                    nc.scalar.activation(lnd[:], dps[:], AF.Ln)
                    ht = h_pool.tile([128, F], BF16, tag="h")
                    nc.scalar.activation(
                        ht[:], lnd[:], AF.Exp, bias=exp_bias[:], scale=-1.0
                    )

                    tt = t_pool.tile([128, F], BF16, tag="t")
                    nc.vector.tensor_tensor(tt[:], kt[:], ht[:], OP.mult)
                    ct = c_pool.tile([128, F], BF16, tag="c")
                    nc.vector.tensor_tensor(ct[:], tt[:], act[:], OP.mult)
                    at = a_pool.tile([128, F], BF16, tag="a")
                    nc.scalar.activation(at[:], ct[:], AF.Copy, bias=0.625, scale=-1.0)
                    bt = b_pool.tile([128, F], BF16, tag="b")
                    nc.vector.tensor_tensor(bt[:], ct[:], et[:], OP.mult)

                    if pending is not None:
                        emit_tail(pending)
                    pending = (tci, o, g, at, bt)
        emit_tail(pending)

    nc.compile()
    return nc


_NC_CACHE: list = []


def _to_bf16(a: np.ndarray) -> np.ndarray:
    import ml_dtypes

    return np.ascontiguousarray(a.astype(ml_dtypes.bfloat16))


def build_in_maps(u_pre: np.ndarray, k_syn: np.ndarray, e_syn: np.ndarray) -> list:
    import ml_dtypes

    eye = np.eye(128, dtype=ml_dtypes.bfloat16)
    neye = (-np.eye(128)).astype(ml_dtypes.bfloat16)
    in_maps = []
    for i in range(N_CORES):
        lo, hi = i * NL, (i + 1) * NL
        in_maps.append(
            {
                "u": _to_bf16(u_pre[lo:hi, 0, :]),
                "k": _to_bf16(k_syn[lo:hi]),
                "e": _to_bf16(e_syn[lo:hi]),
                "ident": eye,
                "nident": neye,
            }
        )
    return in_maps


def kernel(u_pre: np.ndarray, k_syn: np.ndarray, e_syn: np.ndarray) -> np.ndarray:
    if not _NC_CACHE:
        _NC_CACHE.append(_build_nc())
    nc = _NC_CACHE[0]

    in_maps = build_in_maps(u_pre, k_syn, e_syn)
    res = run_bass_kernel_spmd(nc, in_maps, list(range(N_CORES)))
    partials = np.stack([res.results[i]["out"] for i in range(N_CORES)])
    return partials.sum(axis=0, dtype=np.float32)
